# revision 46
# baseline (speedup 1.0000x reference)
"""Trainium2 Bass kernel for nn_DenseTensor (dense_mlp, bilinear form).

Computes out = x @ W + einsum('bd,due,be->bu', x, V, x) + b with
B=1024, D=U=E=512 on 8 NeuronCores.

Algorithm: the quadratic form depends only on the symmetric part of V
in (d,e), so the D*E contraction folds to D(D+1)/2 unordered pairs
enumerated by circulant offset o: pair (d, e=(d+o)%512) for o in
[0,256].  Host pre-folds coefficients Sh[(o,d),u] = V[d,u,e]+V[e,u,d]
(o=0 diag: V[d,u,d]).  This HALVES the PE FLOPs vs the naive einsum.

Sharding: by contraction - core c owns offsets o in [32c, 32c+32);
every core computes a partial full [U,B] output and the host sums the
8 partials (the unshard step for contraction sharding).  Leftover work
rides one extra single-chunk "mini" unit per core, SPMD-uniform:
  cores 0-3 : linear term chunk  (gMini = x^T rows, coeff = W rows)
  core  4   : bias as rank-1     (gMini = ones, coeff row0 = b)
  core  5   : idle (zero coeffs)
  cores 6,7 : the o=256 half-offset pairs

Mixed precision: 14 of each core's 32 offset-units run as double-
pumped fp8 DoubleRow matmuls (2 contraction rows/cycle - half the PE
time of bf16).  Their folded coefficients are host-quantized to
e4m3 after a x256 pre-scale (pow2, undone in the host sum) so the
~N(0, 2/D^2) values sit in e4m3's normal range, and their pair
products are host-staged e4m3 (exactly what the DVE would produce,
at half the DMA bytes and zero DVE work).  Error budget: rel_l2
grows ~0.028*sqrt(NF8/32); NF8=14 measures 1.62e-2 vs the 2e-2
gate (bf16-only is 2.7e-3).  fp8 units sit where the pipeline has
depth (ramp units 1-2 are fully host-staged; the rest alternate or
sit late); putting them too early collapses the DMA lead and costs
a multi-us resync stall.

Per-core, per bf16 o-unit:
  DMA : xw = 512-row window of the wrap-extended x^T at offset o
        (contiguous slabs - no on-device shuffles) + coeff block.
  DVE : G = xT .* xw  (pair products, one bf16 tensor_mul, 2x mode)
  PE  : 4 k-chunks x 4 u-blocks x 2 b-halves matmuls (N=512)
        accumulating into 8 PSUM banks across the whole kernel.
fp8 o-unit: one 512KB g8 DMA + 256KB sh8 DMA, then 2 chunk-pairs x
  4 u-blocks x 2 b-halves DoubleRow matmuls ([128,2,128]x[128,2,512]).

Ramp: 9 zero-weight matmuls gated only on gpsimd memsets issue at
~8.3us (the TileContext prologue owns 0-7.5us) and warm the HAM
clock gate (cold PE runs at 1.2GHz; it opens to 2.4GHz after ~3.4us
of sustained activity) while the first DMA completion receipts
(~2us after transfer) land; every bank's real stream re-opens with
start=True so the zeros are discarded.  Tail: last unit loops
accumulator-major so PSUM banks retire staggered; evac copies
(f32->bf16, halving the store) alternate Vector/Scalar, store DMAs
alternate sync/scalar rings, and the last accumulator goes out in
two 256-col strips so the final receipt-dominated DMA is small.

Measured: 191.6us (baseline 243.6us), zero matmul-stream gaps
>470ns; stream = 817 matmuls at the warm 215.8ns issue cadence.
"""

import sys
import types

import numpy as np
import ml_dtypes

B, D, U = 1024, 512, 512
N_CORES = 8
P = 128                  # partitions
KC = D // P              # k-chunks per unit = 4
NJ = 32                  # o-units per core
NJG = 3                  # host-staged (pair-product) bf16 ramp units
# DoubleRow fp8 units, alternating through the back half so DMA/DVE
# demand stays smooth; unit 31 stays bf16 for the retirement tail.
FP8_UNITS = (1, 2, 12, 14, 16, 18, 20, 22, 24, 26, 27, 28, 29, 30)
FP8_SCALE = 256.0        # coefficient pre-scale (pow2; undone on host)

BF16 = ml_dtypes.bfloat16
F8E4 = ml_dtypes.float8_e4m3     # TRN FP8_EXP4 (bias 7, max 240)


def _ensure_axon_hooks():
    """Provide the antenv.axon_hooks registry if the image lacks it."""
    try:
        import antenv.axon_hooks  # noqa: F401
        return
    except ImportError:
        pass
    mod = types.ModuleType("antenv.axon_hooks")
    mod._hook = None

    def set_axon_ntff_profile_hook(h):
        mod._hook = h

    def get_axon_ntff_profile_hook():
        return mod._hook

    mod.set_axon_ntff_profile_hook = set_axon_ntff_profile_hook
    mod.get_axon_ntff_profile_hook = get_axon_ntff_profile_hook
    sys.modules["antenv.axon_hooks"] = mod
    try:
        import antenv
        antenv.axon_hooks = mod
    except ImportError:
        pass
    try:
        from trn_agent_boot.trn_boot import _ntff_profile_via_ctypes
        hook = _ntff_profile_via_ctypes("/opt/axon/libaxon_pjrt.so")
        if hook is not None:
            set_axon_ntff_profile_hook(hook)
    except Exception:
        pass


def _split_multi_waits(nc, mybir, max_waits=1):
    """Legalize for walrus builds that allow only one sync wait per
    instruction: move extra waits onto same-engine NoOps placed just
    before the offending instruction (queues are in-order, so this is
    semantics-preserving)."""
    for f in nc.m.functions:
        for blk in f.blocks:
            new_insts, changed = [], False
            for inst in blk.instructions:
                si = inst.sync_info
                if si is not None and len(si.on_wait) > max_waits:
                    waits = list(si.on_wait)
                    extra, keep = waits[:-max_waits], waits[-max_waits:]
                    for j, w in enumerate(extra):
                        new_insts.append(mybir.InstNoOp(
                            name=f"{inst.name}-sw{j}",
                            engine=inst.engine,
                            bass_nofuse=True,
                            sync_info=mybir.SyncInfo(on_wait=[w], on_update=[]),
                        ))
                    inst.sync_info = mybir.SyncInfo(
                        on_wait=keep, on_update=list(si.on_update))
                    changed = True
                new_insts.append(inst)
            if changed:
                blk.instructions = new_insts


def _build_program():
    import concourse.bass as bass
    import concourse.mybir as mybir
    import concourse.tile as tile

    f32 = mybir.dt.float32
    bf16 = mybir.dt.bfloat16
    f8 = mybir.dt.float8e4
    DR = mybir.MatmulPerfMode.DoubleRow
    Copy = mybir.ActivationFunctionType.Copy

    nc = bass.Bass(trn_type="TRN2")
    xTc = nc.dram_tensor("xTc", [P, KC, B], bf16, kind="ExternalInput")
    xE = nc.dram_tensor("xE", [544, B], bf16, kind="ExternalInput")
    Sh = nc.dram_tensor("Sh", [NJ, P, KC, U], bf16, kind="ExternalInput")
    Sh8 = nc.dram_tensor("Sh8", [len(FP8_UNITS), P, KC, U], f8,
                         kind="ExternalInput")
    g8d = nc.dram_tensor("g8d", [len(FP8_UNITS), P, KC, B], f8,
                         kind="ExternalInput")
    gMini = nc.dram_tensor("gMini", [P, B], bf16, kind="ExternalInput")
    g01 = nc.dram_tensor("g01", [NJG, P, KC, B], bf16, kind="ExternalInput")
    shMini = nc.dram_tensor("shMini", [P, U], bf16, kind="ExternalInput")
    outs = nc.dram_tensor("outs", [U, B], bf16, kind="ExternalOutput")

    with tile.TileContext(nc) as tc:
        with tc.tile_pool(name="const", bufs=1) as cpool:
            xT_sb = cpool.tile([P, KC, B], bf16)
            out_sb = cpool.tile([P, KC, B], bf16)
            ms_sb = cpool.tile([P, U], bf16)
            gm_sb = cpool.tile([P, B], bf16)
            zW = cpool.tile([P, P], bf16)
            zR = cpool.tile([P, 512], bf16)

            # mini unit first: tiny loads so the PE starts almost
            # immediately while the big unit-0 windows stream in.
            nc.sync.dma_start(out=gm_sb, in_=gMini[:, :])
            nc.scalar.dma_start(out=ms_sb, in_=shMini[:, :])
            # gpsimd memsets finish ~7.9us (nothing else queues on
            # gpsimd), unblocking the warm-up matmuls at ~8.3us.
            nc.gpsimd.memset(zW, 0.0)
            nc.gpsimd.memset(zR, 0.0)

            with tc.tile_pool(name="wp", bufs=6) as wpool, \
                 tc.tile_pool(name="gp", bufs=5) as gpool, \
                 tc.tile_pool(name="sp", bufs=8) as spool, \
                 tc.tile_pool(name="gp8", bufs=4) as gpool8, \
                 tc.tile_pool(name="sp8", bufs=4) as spool8, \
                 tc.tile_pool(name="ap", bufs=1, space="PSUM") as apool:
                accs = [[None, None] for _ in range(4)]
                for ub in range(4):
                    for h in range(2):
                        acc_t = apool.tile([P, 512], f32, tag=f"acc{ub}_{h}")
                        accs[ub][h] = acc_t

                # HAM warm-up: zero matmuls gated only on the gpsimd
                # memsets issue at ~8.3us and keep the HAM clock gate
                # warming (it opens after ~3.4us of sustained
                # activity) while the first real operands stream in.
                # Every bank's real stream re-opens with start=True,
                # so whatever they accumulate is discarded.
                for i in range(9):
                    nc.tensor.matmul(
                        accs[i % 8 // 2][i % 2], zW, zR,
                        start=True, stop=True)
                for ub in range(4):
                    for h in range(2):
                        nc.tensor.matmul(
                            accs[ub][h],
                            ms_sb[:, ub * P:(ub + 1) * P],
                            gm_sb[:, h * 512:(h + 1) * 512],
                            start=True, stop=False)

                # Window chunks k=0..2 ride the sync HWDGE ring,
                # k=3 + coefficients the scalar ring, x^T the gpsimd
                # ring (~25MB per HWDGE ring).  The first two units
                # stay per-k-granular so the PE ramps without waiting
                # for whole tiles; later units use batched DMAs to
                # keep the sequencers' descriptor-gen load low.
                first_computed = min(
                    j for j in range(NJ)
                    if j >= NJG and j not in FP8_UNITS)
                idx8 = 0
                for j in range(NJ):
                    is8 = j in FP8_UNITS
                    if is8:
                        # DoubleRow fp8 unit: coefficients AND pair
                        # products are host-staged e4m3 (half the
                        # bytes of the window path, no DVE work, so
                        # nothing can head-of-line-block the strict-
                        # FIFO vector queue); each matmul contracts a
                        # pair of 128-row chunks in one 512-cycle
                        # stream.
                        g = gpool8.tile([P, KC, B], f8, tag="g8")
                        sh = spool8.tile([P, KC, U], f8, tag="sh8")
                        nc.scalar.dma_start(out=sh, in_=Sh8[idx8])
                        nc.sync.dma_start(out=g, in_=g8d[idx8])
                        idx8 += 1
                    else:
                        g = gpool.tile([P, KC, B], bf16, tag="g")
                        sh = spool.tile([P, KC, U], bf16, tag="sh")
                    if is8:
                        pass
                    elif j < NJG:
                        # first units: host-staged pair products (same
                        # bytes as their windows) - no TT, no xT
                        # dependency on the critical ramp.
                        for k in range(KC):
                            nc.sync.dma_start(
                                out=g[:, k, :], in_=g01[j, :, k, :])
                            nc.scalar.dma_start(
                                out=sh[:, k, :], in_=Sh[j, :, k, :])
                    else:
                        if j == first_computed:
                            for k in range(KC):
                                nc.scalar.dma_start(
                                    out=xT_sb[:, k, :], in_=xTc[:, k, :])
                        xw = wpool.tile([P, KC, B], bf16, tag="xw")
                        nc.scalar.dma_start(out=sh, in_=Sh[j])
                        for k in range(KC):
                            eng = nc.sync if k < 3 else nc.scalar
                            eng.dma_start(
                                out=xw[:, k, :],
                                in_=xE[j + P * k: j + P * (k + 1), :])
                            nc.vector.tensor_mul(
                                g[:, k, :], xT_sb[:, k, :], xw[:, k, :])
                    if is8:
                        for t in range(2):
                            for ub in range(4):
                                for h in range(2):
                                    nc.tensor.matmul(
                                        accs[ub][h],
                                        sh[:, 2 * t:2 * t + 2,
                                           ub * P:(ub + 1) * P],
                                        g[:, 2 * t:2 * t + 2,
                                          h * 512:(h + 1) * 512],
                                        start=False, stop=False,
                                        perf_mode=DR)
                    elif j < NJ - 1:
                        for k in range(KC):
                            for ub in range(4):
                                for h in range(2):
                                    nc.tensor.matmul(
                                        accs[ub][h],
                                        sh[:, k, ub * P:(ub + 1) * P],
                                        g[:, k, h * 512:(h + 1) * 512],
                                        start=False, stop=False)
                    else:
                        # last unit: accumulator-major so PSUM banks
                        # retire staggered and evac overlaps the tail.
                        for ub in range(4):
                            for h in range(2):
                                for k in range(KC):
                                    nc.tensor.matmul(
                                        accs[ub][h],
                                        sh[:, k, ub * P:(ub + 1) * P],
                                        g[:, k, h * 512:(h + 1) * 512],
                                        start=False, stop=(k == KC - 1))

                # Evac: copies alternate Vector/Scalar, the store DMAs
                # alternate sync/scalar rings, and the last accumulator
                # goes out in 256-col strips so the final (completion-
                # latency-dominated) DMA is as small and early as
                # possible.
                outs_r = outs.rearrange("(ub p) b -> p ub b", p=P)
                i = 0
                for ub in range(4):
                    for h in range(2):
                        nsplit = 2 if i == 7 else 1
                        w = 512 // nsplit
                        for s in range(nsplit):
                            c0 = h * 512 + s * w
                            dst = out_sb[:, ub, c0:c0 + w]
                            src = accs[ub][h][:, s * w:(s + 1) * w]
                            if (i + s) % 2 == 0:
                                nc.vector.tensor_copy(dst, src)
                            else:
                                nc.scalar.activation(dst, src, Copy)
                            deng = nc.sync if (i + s) % 2 == 0 else nc.scalar
                            deng.dma_start(
                                out=outs_r[:, ub, c0:c0 + w], in_=dst)
                        i += 1

    _split_multi_waits(nc, mybir, max_waits=1)
    return nc


def _host_inputs(x, W, V, b):
    """Build the per-core input arrays (all host-side prep)."""
    xT_bf = np.ascontiguousarray(x.T).astype(BF16)          # [D, B]
    xT_ext = np.concatenate([xT_bf, xT_bf[:256]], axis=0)   # [768, B]
    xTc_np = np.ascontiguousarray(
        xT_bf.reshape(KC, P, B).transpose(1, 0, 2))         # [P, KC, B]

    # folded symmetric coefficients
    Vt = V.transpose(0, 2, 1)                               # [d, e, u]
    Ssum = Vt + Vt.transpose(1, 0, 2)                       # V[d,u,e]+V[e,u,d]
    dd = np.arange(D)
    Vdiag = V[dd, :, dd]                                    # [d, u]

    ones = np.ones((P, B), dtype=BF16)
    zeros = np.zeros((P, B), dtype=BF16)

    def unit_block(M):      # [d, u] -> [p, k, u]
        return M.reshape(KC, P, U).transpose(1, 0, 2)

    in_maps = []
    for c in range(N_CORES):
        # All stationary coefficients are pre-scaled by FP8_SCALE
        # (power of two - exact in bf16) so the fp8 units' e4m3
        # coefficients sit in the format's normal range; the host
        # divides the summed partials back down.
        Sh_np = np.zeros((NJ, P, KC, U), dtype=np.float32)
        Sh8_np = np.zeros((len(FP8_UNITS), P, KC, U), dtype=np.float32)
        i8 = 0
        for j in range(NJ):
            o = 32 * c + j
            M = Vdiag if o == 0 else Ssum[dd, (dd + o) % D, :]
            Sh_np[j] = unit_block(M) * FP8_SCALE
            if j in FP8_UNITS:
                Sh8_np[i8] = Sh_np[j]
                i8 += 1

        mini_s = np.zeros((P, U), dtype=np.float32)
        if c < 4:                       # linear term, chunk c
            gm = xT_bf[P * c: P * (c + 1)]
            mini_s = W[P * c: P * (c + 1), :].astype(np.float32)
        elif c == 4:                    # bias as rank-1 with ones
            gm = ones
            mini_s[0, :] = b
        elif c == 5:                    # idle
            gm = zeros
        else:                           # o=256 pairs, halves on 6 and 7
            d0 = P * (c - 6)
            gm = (xT_bf[d0: d0 + P] * xT_bf[d0 + 256: d0 + 256 + P]
                  ).astype(BF16)
            mini_s = Ssum[dd[d0:d0 + P], dd[d0:d0 + P] + 256, :]
        mini_s = mini_s * FP8_SCALE

        xE_c = np.ascontiguousarray(xT_ext[32 * c: 32 * c + 544])
        g01_c = np.empty((NJG, P, KC, B), dtype=BF16)
        for j in range(NJG):
            for k in range(KC):
                g01_c[j, :, k, :] = (
                    xTc_np[:, k, :] * xE_c[j + P * k: j + P * (k + 1)])
        g8_c = np.empty((len(FP8_UNITS), P, KC, B), dtype=F8E4)
        for i8, j in enumerate(FP8_UNITS):
            for k in range(KC):
                g8_c[i8, :, k, :] = (
                    xTc_np[:, k, :].astype(np.float32)
                    * xE_c[j + P * k: j + P * (k + 1)].astype(np.float32)
                ).astype(F8E4)

        in_maps.append({
            "xTc": xTc_np,
            "xE": xE_c,
            "Sh": Sh_np.astype(BF16),
            "Sh8": Sh8_np.astype(F8E4),
            "g8d": g8_c,
            "gMini": np.ascontiguousarray(gm),
            "g01": g01_c,
            "shMini": mini_s.astype(BF16),
        })
    return in_maps


_LAST_RUN = {}


def kernel(x, W, V, b):
    _ensure_axon_hooks()
    import concourse.bass_utils as bass_utils
    bass_utils.upload_artifacts = lambda d: f"local:{d}"

    x = np.asarray(x, dtype=np.float32)
    W = np.asarray(W, dtype=np.float32)
    V = np.asarray(V, dtype=np.float32)
    b = np.asarray(b, dtype=np.float32)

    in_maps = _host_inputs(x, W, V, b)

    nc = _build_program()
    res = None
    last_exc = None
    for attempt in range(3):
        try:
            res = bass_utils.run_bass_kernel_spmd(
                nc, in_maps, core_ids=list(range(N_CORES)))
            break
        except Exception as e:  # transient NRT device errors have been seen
            last_exc = e
    if res is None:
        raise last_exc
    _LAST_RUN["result"] = res

    acc = np.zeros((U, B), dtype=np.float64)
    for c in range(N_CORES):
        acc += np.asarray(res.results[c]["outs"], dtype=np.float64)
    acc *= 1.0 / FP8_SCALE
    return np.ascontiguousarray(acc.T).astype(np.float32)



# revision 48
# speedup vs baseline: 1.0347x; 1.0347x over previous
"""Trainium2 Bass kernel for nn_DenseTensor (dense_mlp, bilinear form).

Computes out = x @ W + einsum('bd,due,be->bu', x, V, x) + b with
B=1024, D=U=E=512 on 8 NeuronCores.

Algorithm: the quadratic form depends only on the symmetric part of V
in (d,e), so the D*E contraction folds to D(D+1)/2 unordered pairs
enumerated by circulant offset o: pair (d, e=(d+o)%512) for o in
[0,256].  Host pre-folds coefficients Sh[(o,d),u] = V[d,u,e]+V[e,u,d]
(o=0 diag: V[d,u,d]).  This HALVES the PE FLOPs vs the naive einsum.

Sharding: by contraction - core c owns offsets o in [32c, 32c+32);
every core computes a partial full [U,B] output and the host sums the
8 partials (the unshard step for contraction sharding).  Leftover work
rides one extra single-chunk "mini" unit per core, SPMD-uniform:
  cores 0-3 : linear term chunk  (gMini = x^T rows, coeff = W rows)
  core  4   : bias as rank-1     (gMini = ones, coeff row0 = b)
  core  5   : idle (zero coeffs)
  cores 6,7 : the o=256 half-offset pairs

Mixed precision: 14 of each core's 32 offset-units run as double-
pumped fp8 DoubleRow matmuls (2 contraction rows/cycle - half the PE
time of bf16).  Their folded coefficients are host-quantized to
e4m3 after a x256 pre-scale (pow2, undone in the host sum) so the
~N(0, 2/D^2) values sit in e4m3's normal range, and their pair
products are host-staged e4m3 (exactly what the DVE would produce,
at half the DMA bytes and zero DVE work).  Error budget: rel_l2
grows ~0.028*sqrt(NF8/32); NF8=14 measures 1.62e-2 vs the 2e-2
gate (bf16-only is 2.7e-3).  fp8 units sit where the pipeline has
depth (ramp units 1-2 are fully host-staged; the rest alternate or
sit late); putting them too early collapses the DMA lead and costs
a multi-us resync stall.

Per-core, per bf16 o-unit:
  DMA : xw = 512-row window of the wrap-extended x^T at offset o
        (contiguous slabs - no on-device shuffles) + coeff block.
  DVE : G = xT .* xw  (pair products, one bf16 tensor_mul, 2x mode)
  PE  : 4 k-chunks x 4 u-blocks x 2 b-halves matmuls (N=512)
        accumulating into 8 PSUM banks across the whole kernel.
fp8 o-unit: one 512KB g8 DMA + 256KB sh8 DMA, then 2 chunk-pairs x
  4 u-blocks x 2 b-halves DoubleRow matmuls ([128,2,128]x[128,2,512]).

Ramp: 9 zero-weight matmuls gated only on gpsimd memsets issue at
~8.3us (the TileContext prologue owns 0-7.5us) and warm the HAM
clock gate (cold PE runs at 1.2GHz; it opens to 2.4GHz after ~3.4us
of sustained activity) while the first DMA completion receipts
(~2us after transfer) land; every bank's real stream re-opens with
start=True so the zeros are discarded.  Tail: last unit loops
accumulator-major so PSUM banks retire staggered; evac copies
(f32->bf16, halving the store) alternate Vector/Scalar, store DMAs
alternate sync/scalar rings, and the last accumulator goes out in
two 256-col strips so the final receipt-dominated DMA is small.

Measured: 191.6us (baseline 243.6us), zero matmul-stream gaps
>470ns; stream = 817 matmuls at the warm 215.8ns issue cadence.
"""

import sys
import types

import numpy as np
import ml_dtypes

B, D, U = 1024, 512, 512
N_CORES = 8
P = 128                  # partitions
KC = D // P              # k-chunks per unit = 4
NJ = 32                  # o-units per core
NJG = 3                  # host-staged (pair-product) bf16 ramp units
# DoubleRow fp8 units, alternating through the back half so DMA/DVE
# demand stays smooth; unit 31 stays bf16 for the retirement tail.
FP8_UNITS = (1, 2, 12, 14, 16, 18, 20, 22, 23, 24, 25, 26, 27, 28, 29, 30)
FP8_SCALE = 256.0        # coefficient pre-scale (pow2; undone on host)

BF16 = ml_dtypes.bfloat16
F8E4 = ml_dtypes.float8_e4m3     # TRN FP8_EXP4 (bias 7, max 240)


def _ensure_axon_hooks():
    """Provide the antenv.axon_hooks registry if the image lacks it."""
    try:
        import antenv.axon_hooks  # noqa: F401
        return
    except ImportError:
        pass
    mod = types.ModuleType("antenv.axon_hooks")
    mod._hook = None

    def set_axon_ntff_profile_hook(h):
        mod._hook = h

    def get_axon_ntff_profile_hook():
        return mod._hook

    mod.set_axon_ntff_profile_hook = set_axon_ntff_profile_hook
    mod.get_axon_ntff_profile_hook = get_axon_ntff_profile_hook
    sys.modules["antenv.axon_hooks"] = mod
    try:
        import antenv
        antenv.axon_hooks = mod
    except ImportError:
        pass
    try:
        from trn_agent_boot.trn_boot import _ntff_profile_via_ctypes
        hook = _ntff_profile_via_ctypes("/opt/axon/libaxon_pjrt.so")
        if hook is not None:
            set_axon_ntff_profile_hook(hook)
    except Exception:
        pass


def _split_multi_waits(nc, mybir, max_waits=1):
    """Legalize for walrus builds that allow only one sync wait per
    instruction: move extra waits onto same-engine NoOps placed just
    before the offending instruction (queues are in-order, so this is
    semantics-preserving)."""
    for f in nc.m.functions:
        for blk in f.blocks:
            new_insts, changed = [], False
            for inst in blk.instructions:
                si = inst.sync_info
                if si is not None and len(si.on_wait) > max_waits:
                    waits = list(si.on_wait)
                    extra, keep = waits[:-max_waits], waits[-max_waits:]
                    for j, w in enumerate(extra):
                        new_insts.append(mybir.InstNoOp(
                            name=f"{inst.name}-sw{j}",
                            engine=inst.engine,
                            bass_nofuse=True,
                            sync_info=mybir.SyncInfo(on_wait=[w], on_update=[]),
                        ))
                    inst.sync_info = mybir.SyncInfo(
                        on_wait=keep, on_update=list(si.on_update))
                    changed = True
                new_insts.append(inst)
            if changed:
                blk.instructions = new_insts


def _build_program():
    import concourse.bass as bass
    import concourse.mybir as mybir
    import concourse.tile as tile

    f32 = mybir.dt.float32
    bf16 = mybir.dt.bfloat16
    f8 = mybir.dt.float8e4
    DR = mybir.MatmulPerfMode.DoubleRow
    Copy = mybir.ActivationFunctionType.Copy

    nc = bass.Bass(trn_type="TRN2")
    xTc = nc.dram_tensor("xTc", [P, KC, B], bf16, kind="ExternalInput")
    xE = nc.dram_tensor("xE", [544, B], bf16, kind="ExternalInput")
    Sh = nc.dram_tensor("Sh", [NJ, P, KC, U], bf16, kind="ExternalInput")
    Sh8 = nc.dram_tensor("Sh8", [len(FP8_UNITS), P, KC, U], f8,
                         kind="ExternalInput")
    g8d = nc.dram_tensor("g8d", [len(FP8_UNITS), P, KC, B], f8,
                         kind="ExternalInput")
    gMini = nc.dram_tensor("gMini", [P, B], bf16, kind="ExternalInput")
    g01 = nc.dram_tensor("g01", [NJG, P, KC, B], bf16, kind="ExternalInput")
    shMini = nc.dram_tensor("shMini", [P, U], bf16, kind="ExternalInput")
    outs = nc.dram_tensor("outs", [U, B], bf16, kind="ExternalOutput")

    with tile.TileContext(nc) as tc:
        with tc.tile_pool(name="const", bufs=1) as cpool:
            xT_sb = cpool.tile([P, KC, B], bf16)
            out_sb = cpool.tile([P, KC, B], bf16)
            ms_sb = cpool.tile([P, U], bf16)
            gm_sb = cpool.tile([P, B], bf16)
            zW = cpool.tile([P, P], bf16)
            zR = cpool.tile([P, 512], bf16)

            # mini unit first: tiny loads so the PE starts almost
            # immediately while the big unit-0 windows stream in.
            nc.sync.dma_start(out=gm_sb, in_=gMini[:, :])
            nc.scalar.dma_start(out=ms_sb, in_=shMini[:, :])
            # gpsimd memsets finish ~7.9us (nothing else queues on
            # gpsimd), unblocking the warm-up matmuls at ~8.3us.
            nc.gpsimd.memset(zW, 0.0)
            nc.gpsimd.memset(zR, 0.0)

            with tc.tile_pool(name="wp", bufs=6) as wpool, \
                 tc.tile_pool(name="gp", bufs=5) as gpool, \
                 tc.tile_pool(name="sp", bufs=8) as spool, \
                 tc.tile_pool(name="gp8", bufs=5) as gpool8, \
                 tc.tile_pool(name="sp8", bufs=5) as spool8, \
                 tc.tile_pool(name="ap", bufs=1, space="PSUM") as apool:
                accs = [[None, None] for _ in range(4)]
                for ub in range(4):
                    for h in range(2):
                        acc_t = apool.tile([P, 512], f32, tag=f"acc{ub}_{h}")
                        accs[ub][h] = acc_t

                # HAM warm-up: zero matmuls gated only on the gpsimd
                # memsets issue at ~8.3us and keep the HAM clock gate
                # warming (it opens after ~3.4us of sustained
                # activity) while the first real operands stream in.
                # Every bank's real stream re-opens with start=True,
                # so whatever they accumulate is discarded.
                for i in range(9):
                    nc.tensor.matmul(
                        accs[i % 8 // 2][i % 2], zW, zR,
                        start=True, stop=True)
                for ub in range(4):
                    for h in range(2):
                        nc.tensor.matmul(
                            accs[ub][h],
                            ms_sb[:, ub * P:(ub + 1) * P],
                            gm_sb[:, h * 512:(h + 1) * 512],
                            start=True, stop=False)

                # Window chunks k=0..2 ride the sync HWDGE ring,
                # k=3 + coefficients the scalar ring, x^T the gpsimd
                # ring (~25MB per HWDGE ring).  The first two units
                # stay per-k-granular so the PE ramps without waiting
                # for whole tiles; later units use batched DMAs to
                # keep the sequencers' descriptor-gen load low.
                first_computed = min(
                    j for j in range(NJ)
                    if j >= NJG and j not in FP8_UNITS)
                idx8 = 0
                for j in range(NJ):
                    is8 = j in FP8_UNITS
                    if is8:
                        # DoubleRow fp8 unit: coefficients AND pair
                        # products are host-staged e4m3 (half the
                        # bytes of the window path, no DVE work, so
                        # nothing can head-of-line-block the strict-
                        # FIFO vector queue); each matmul contracts a
                        # pair of 128-row chunks in one 512-cycle
                        # stream.
                        g = gpool8.tile([P, KC, B], f8, tag="g8")
                        sh = spool8.tile([P, KC, U], f8, tag="sh8")
                        nc.scalar.dma_start(out=sh, in_=Sh8[idx8])
                        nc.sync.dma_start(out=g, in_=g8d[idx8])
                        idx8 += 1
                    else:
                        g = gpool.tile([P, KC, B], bf16, tag="g")
                        sh = spool.tile([P, KC, U], bf16, tag="sh")
                    if is8:
                        pass
                    elif j < NJG:
                        # first units: host-staged pair products (same
                        # bytes as their windows) - no TT, no xT
                        # dependency on the critical ramp.
                        for k in range(KC):
                            nc.sync.dma_start(
                                out=g[:, k, :], in_=g01[j, :, k, :])
                            nc.scalar.dma_start(
                                out=sh[:, k, :], in_=Sh[j, :, k, :])
                    else:
                        if j == first_computed:
                            for k in range(KC):
                                nc.scalar.dma_start(
                                    out=xT_sb[:, k, :], in_=xTc[:, k, :])
                        xw = wpool.tile([P, KC, B], bf16, tag="xw")
                        nc.scalar.dma_start(out=sh, in_=Sh[j])
                        for k in range(KC):
                            eng = nc.sync if k < 3 else nc.scalar
                            eng.dma_start(
                                out=xw[:, k, :],
                                in_=xE[j + P * k: j + P * (k + 1), :])
                            nc.vector.tensor_mul(
                                g[:, k, :], xT_sb[:, k, :], xw[:, k, :])
                    if is8:
                        for t in range(2):
                            for ub in range(4):
                                for h in range(2):
                                    nc.tensor.matmul(
                                        accs[ub][h],
                                        sh[:, 2 * t:2 * t + 2,
                                           ub * P:(ub + 1) * P],
                                        g[:, 2 * t:2 * t + 2,
                                          h * 512:(h + 1) * 512],
                                        start=False, stop=False,
                                        perf_mode=DR)
                    elif j < NJ - 1:
                        for k in range(KC):
                            for ub in range(4):
                                for h in range(2):
                                    nc.tensor.matmul(
                                        accs[ub][h],
                                        sh[:, k, ub * P:(ub + 1) * P],
                                        g[:, k, h * 512:(h + 1) * 512],
                                        start=False, stop=False)
                    else:
                        # last unit: accumulator-major so PSUM banks
                        # retire staggered and evac overlaps the tail.
                        for ub in range(4):
                            for h in range(2):
                                for k in range(KC):
                                    nc.tensor.matmul(
                                        accs[ub][h],
                                        sh[:, k, ub * P:(ub + 1) * P],
                                        g[:, k, h * 512:(h + 1) * 512],
                                        start=False, stop=(k == KC - 1))

                # Evac: copies alternate Vector/Scalar, the store DMAs
                # alternate sync/scalar rings, and the last accumulator
                # goes out in 256-col strips so the final (completion-
                # latency-dominated) DMA is as small and early as
                # possible.
                outs_r = outs.rearrange("(ub p) b -> p ub b", p=P)
                i = 0
                for ub in range(4):
                    for h in range(2):
                        nsplit = 2 if i == 7 else 1
                        w = 512 // nsplit
                        for s in range(nsplit):
                            c0 = h * 512 + s * w
                            dst = out_sb[:, ub, c0:c0 + w]
                            src = accs[ub][h][:, s * w:(s + 1) * w]
                            if (i + s) % 2 == 0:
                                nc.vector.tensor_copy(dst, src)
                            else:
                                nc.scalar.activation(dst, src, Copy)
                            deng = nc.sync if (i + s) % 2 == 0 else nc.scalar
                            deng.dma_start(
                                out=outs_r[:, ub, c0:c0 + w], in_=dst)
                        i += 1

    _split_multi_waits(nc, mybir, max_waits=1)
    return nc


def _host_inputs(x, W, V, b):
    """Build the per-core input arrays (all host-side prep)."""
    xT_bf = np.ascontiguousarray(x.T).astype(BF16)          # [D, B]
    xT_ext = np.concatenate([xT_bf, xT_bf[:256]], axis=0)   # [768, B]
    xTc_np = np.ascontiguousarray(
        xT_bf.reshape(KC, P, B).transpose(1, 0, 2))         # [P, KC, B]

    # folded symmetric coefficients
    Vt = V.transpose(0, 2, 1)                               # [d, e, u]
    Ssum = Vt + Vt.transpose(1, 0, 2)                       # V[d,u,e]+V[e,u,d]
    dd = np.arange(D)
    Vdiag = V[dd, :, dd]                                    # [d, u]

    ones = np.ones((P, B), dtype=BF16)
    zeros = np.zeros((P, B), dtype=BF16)

    def unit_block(M):      # [d, u] -> [p, k, u]
        return M.reshape(KC, P, U).transpose(1, 0, 2)

    in_maps = []
    for c in range(N_CORES):
        # All stationary coefficients are pre-scaled by FP8_SCALE
        # (power of two - exact in bf16) so the fp8 units' e4m3
        # coefficients sit in the format's normal range; the host
        # divides the summed partials back down.
        Sh_np = np.zeros((NJ, P, KC, U), dtype=np.float32)
        Sh8_np = np.zeros((len(FP8_UNITS), P, KC, U), dtype=np.float32)
        i8 = 0
        for j in range(NJ):
            o = 32 * c + j
            M = Vdiag if o == 0 else Ssum[dd, (dd + o) % D, :]
            Sh_np[j] = unit_block(M) * FP8_SCALE
            if j in FP8_UNITS:
                Sh8_np[i8] = Sh_np[j]
                i8 += 1

        mini_s = np.zeros((P, U), dtype=np.float32)
        if c < 4:                       # linear term, chunk c
            gm = xT_bf[P * c: P * (c + 1)]
            mini_s = W[P * c: P * (c + 1), :].astype(np.float32)
        elif c == 4:                    # bias as rank-1 with ones
            gm = ones
            mini_s[0, :] = b
        elif c == 5:                    # idle
            gm = zeros
        else:                           # o=256 pairs, halves on 6 and 7
            d0 = P * (c - 6)
            gm = (xT_bf[d0: d0 + P] * xT_bf[d0 + 256: d0 + 256 + P]
                  ).astype(BF16)
            mini_s = Ssum[dd[d0:d0 + P], dd[d0:d0 + P] + 256, :]
        mini_s = mini_s * FP8_SCALE

        xE_c = np.ascontiguousarray(xT_ext[32 * c: 32 * c + 544])
        g01_c = np.empty((NJG, P, KC, B), dtype=BF16)
        for j in range(NJG):
            for k in range(KC):
                g01_c[j, :, k, :] = (
                    xTc_np[:, k, :] * xE_c[j + P * k: j + P * (k + 1)])
        g8_c = np.empty((len(FP8_UNITS), P, KC, B), dtype=F8E4)
        for i8, j in enumerate(FP8_UNITS):
            for k in range(KC):
                g8_c[i8, :, k, :] = (
                    xTc_np[:, k, :].astype(np.float32)
                    * xE_c[j + P * k: j + P * (k + 1)].astype(np.float32)
                ).astype(F8E4)

        in_maps.append({
            "xTc": xTc_np,
            "xE": xE_c,
            "Sh": Sh_np.astype(BF16),
            "Sh8": Sh8_np.astype(F8E4),
            "g8d": g8_c,
            "gMini": np.ascontiguousarray(gm),
            "g01": g01_c,
            "shMini": mini_s.astype(BF16),
        })
    return in_maps


_LAST_RUN = {}


def kernel(x, W, V, b):
    _ensure_axon_hooks()
    import concourse.bass_utils as bass_utils
    bass_utils.upload_artifacts = lambda d: f"local:{d}"

    x = np.asarray(x, dtype=np.float32)
    W = np.asarray(W, dtype=np.float32)
    V = np.asarray(V, dtype=np.float32)
    b = np.asarray(b, dtype=np.float32)

    in_maps = _host_inputs(x, W, V, b)

    nc = _build_program()
    res = None
    last_exc = None
    for attempt in range(3):
        try:
            res = bass_utils.run_bass_kernel_spmd(
                nc, in_maps, core_ids=list(range(N_CORES)))
            break
        except Exception as e:  # transient NRT device errors have been seen
            last_exc = e
    if res is None:
        raise last_exc
    _LAST_RUN["result"] = res

    acc = np.zeros((U, B), dtype=np.float64)
    for c in range(N_CORES):
        acc += np.asarray(res.results[c]["outs"], dtype=np.float64)
    acc *= 1.0 / FP8_SCALE
    return np.ascontiguousarray(acc.T).astype(np.float32)

